# revision 1
# baseline (speedup 1.0000x reference)
"""Trainium2 Bass kernel for nn_BearingQCCFeatureMotion (v5).

Pipeline (B=2, F=8, P=2048, SCALES=(5,15,40)):
  - host (numpy fp32, mirrors the reference formulas): bearing quaternions,
    per-transition relative quaternions q_fwd, squared norms; builds augmented
    matmul tables so the device computes both the pairwise neg-distance matrix
    D[p,q] = 2<x_p,x_q> - |x_p|^2 - |x_q|^2 and the quaternion-dot matrix
    Q[p,q] = <q_fwd[p], q_fwd[q]> as K=5 / K=4 matmuls.
  - device (8 cores, data-parallel over the P dimension; each core handles a
    256-row chunk of all 14 (b,t) transitions = 28 tiles of 128x2048):
      PE   : D and Q matmuls into PSUM
      ACT  : PSUM->SBUF moves (copy/|Q|/Q^2), u = 1/sqrt(max(1-Q^2, 2e-7))
             via Abs_reciprocal_sqrt of the GPSIMD-clamped min(Q^2-1, -2e-7),
             g = arctan((1-|Q|)*u) = arccos(|Q|)/2, and the
             accumulating copies for the k=5/15 masked sums
      DVE  : segment-candidate top-40 (16x max8 over 128-col segments ->
             128 candidates -> exact top-40 of candidates via 5x max8 +
             4x match_replace over 128 cols), the (1-x)*u product, the
             k=40 masked sum, and the margin count at rank 40
      GPSIMD: is_ge masks + mask*g products for k=5/15
    A full-row count of D >= theta40*(1+1e-5)-1e-5 verifies both that the
    segment candidates contained the true top-40 AND that no (near-)ties
    straddle the rank-40 boundary; rank-5/15 boundaries are gap-checked on
    the host from maxv. Flagged rows (~4%) are recomputed exactly on the
    host, vectorized.
  - host: means over transitions, flagged-row repair, lower-median scale,
    exp, broadcast to the (B, 3, F*P) output.
"""

import numpy as np

B, F, P = 2, 8, 2048
T = F - 1
NBT = B * T            # 14 independent (b, t) transitions
NCORES = 8
CH = P // NCORES       # 256 rows per core
SCALES = (5, 15, 40)
AUGK = 5               # [2x, 2y, 2z, sq, 1] . [x', y', z', -1, -sq']
QK = 4

SEG = 16               # stage1 segments per row
SEGW = P // SEG        # 128 columns per segment
KMAX = 40

# Margins: device D vs reference D divergence is a few fp32 ulps of the
# O(10) matmul terms (~3e-6 absolute); 1e-5 gives ~3x headroom.
CNT_REL = 1e-5
CNT_ABS = 1e-5
GAP_REL = 1e-5
GAP_ABS = 1e-5

_PROG = None


def _build_program(repeat=1, hw_loop_n=None):
    """Build + compile the per-core Bass/Tile program (same for all cores).

    hw_loop_n wraps the whole tile sequence in a device-side For_i loop
    (used for slope-based timing; outputs are just overwritten each pass).
    """
    from contextlib import ExitStack
    import concourse.tile as tile
    from concourse import bacc, mybir
    import concourse.bass as bass

    f32 = mybir.dt.float32
    Alu = mybir.AluOpType
    Act = mybir.ActivationFunctionType

    nc = bacc.Bacc("TRN2", target_bir_lowering=False, debug=False)

    # packed per-bt tables: [0:P]=aug_rhs, [P:P+CH]=aug_lhsT,
    # [P+CH:2P+CH]=qf_rhs (rows 0-3), [2P+CH:2P+2CH]=qf_lhsT (rows 0-3)
    tables = nc.dram_tensor("tables", [NBT, AUGK, 2 * (P + CH)], f32,
                            kind="ExternalInput").ap()
    # packed output: cols 0:4 = sums, 4:44 = maxv
    out_o = nc.dram_tensor("out", [NBT, CH, 4 + KMAX], f32,
                           kind="ExternalOutput").ap()

    with tile.TileContext(nc) as tc, ExitStack() as ctx:
        tabs = ctx.enter_context(tc.tile_pool(name="tabs", bufs=2))
        psum_d = ctx.enter_context(tc.tile_pool(name="psd", bufs=1, space="PSUM"))
        psum_q = ctx.enter_context(tc.tile_pool(name="psq", bufs=1, space="PSUM"))
        work2 = ctx.enter_context(tc.tile_pool(name="work", bufs=2))
        work3 = ctx.enter_context(tc.tile_pool(name="work3", bufs=3))
        small = ctx.enter_context(tc.tile_pool(name="small", bufs=3))
        # bufs=3 on long-lived tags measured WORSE under this axon stack's
        # latency regime (3820us vs 2686us slope, though machine-state drift
        # muddies it); keep the measured-best shallow config.
        deep_tags = ()

        def work(shape, dtype, tag):
            pool = work3 if tag in deep_tags else work2
            return pool.tile(shape, dtype, tag=tag, name=tag)

        if hw_loop_n is not None:
            ctx.enter_context(tc.For_i(0, hw_loop_n, 1))

        for bt in [i % NBT for i in range(NBT * repeat)]:
            # per-(b,t) tables, one packed DMA; all matmul operand slices
            # start at partition 0
            tab = tabs.tile([AUGK, 2 * (P + CH)], f32, tag="tab")
            nc.sync.dma_start(tab[:], tables[bt])
            a_rhs = tab[:, 0:P]
            a_lhs = tab[:, P:P + CH]
            q_rhs = tab[0:QK, P + CH:P + CH + P]
            q_lhs = tab[0:QK, 2 * P + CH:2 * P + 2 * CH]

            for ch in range(CH // 128):
                ro = ch * 128

                psd = psum_d.tile([128, P], f32)
                for j in range(4):
                    nc.tensor.matmul(
                        psd[:, bass.ts(j, 512)],
                        lhsT=a_lhs[:, ro:ro + 128],
                        rhs=a_rhs[:, bass.ts(j, 512)],
                        start=True, stop=True)
                psq = psum_q.tile([128, P], f32)
                for j in range(4):
                    nc.tensor.matmul(
                        psq[:, bass.ts(j, 512)],
                        lhsT=q_lhs[:, ro:ro + 128],
                        rhs=q_rhs[:, bass.ts(j, 512)],
                        start=True, stop=True)

                dsb = work([128, P], f32, "dsb")
                nc.scalar.copy(dsb[:], psd[:])
                x = work([128, P], f32, "x")
                nc.scalar.activation(x[:], psq[:], Act.Abs)
                x2 = work([128, P], f32, "x2")
                nc.scalar.activation(x2[:], psq[:], Act.Square)
                # t = min(x^2 - 1, -2e-7) (clamped away from 0);
                # u = 1/sqrt(|t|) = 1/sqrt(max(1 - x^2, 2e-7)).
                # On DVE (not GPSIMD) to keep the geo chain a two-engine
                # ACT<->DVE alternation — GPSIMD only does off-chain masks.
                t = work([128, P], f32, "t")
                nc.vector.tensor_scalar(out=t[:], in0=x2[:], scalar1=1.0,
                                        scalar2=-2e-7, op0=Alu.subtract,
                                        op1=Alu.min)
                u = work([128, P], f32, "u")
                nc.scalar.activation(u[:], t[:], Act.Abs_reciprocal_sqrt)
                # marg = (x-1)*u; geo/4 = g = arctan(-marg)
                marg = work([128, P], f32, "marg")
                nc.vector.scalar_tensor_tensor(
                    out=marg[:], in0=x[:], scalar=1.0, in1=u[:],
                    op0=Alu.subtract, op1=Alu.mult)
                g = work([128, P], f32, "g")
                nc.scalar.activation(g[:], marg[:], Act.Arctan, scale=-1.0)

                outt = small.tile([128, 4 + KMAX], f32, tag="outt")
                maxv = outt[:, 4:4 + KMAX]
                sums = outt[:, 0:4]
                # stage1: top-8 of each 128-col segment -> 128 candidates
                cand = small.tile([128, SEG * 8], f32, tag="cand")
                for s in range(SEG):
                    nc.vector.max(cand[:, 8 * s:8 * s + 8],
                                  dsb[:, SEGW * s:SEGW * (s + 1)])
                # stage2: exact top-40 of the candidates
                candw = small.tile([128, SEG * 8], f32, tag="candw")
                nc.vector.max(maxv[:, 0:8], cand[:])
                nc.vector.match_replace(candw[:], maxv[:, 0:8], cand[:], -1e30)
                for r in range(1, 5):
                    nc.vector.max(maxv[:, 8 * r:8 * r + 8], candw[:])
                    if r < 4:
                        nc.vector.match_replace(candw[:], maxv[:, 8 * r:8 * r + 8],
                                                candw[:], -1e30)

                nc.vector.memset(sums[:], 0.0)
                # k=5 / k=15 masked sums: GPSIMD mask+mult, ACT accumulates
                for i, col in ((0, 4), (1, 14)):
                    mk = work([128, P], f32, f"mk{i}")
                    nc.gpsimd.tensor_scalar(out=mk[:], in0=dsb[:],
                                            scalar1=maxv[:, col:col + 1],
                                            scalar2=None, op0=Alu.is_ge)
                    nc.gpsimd.tensor_tensor(out=mk[:], in0=mk[:], in1=g[:],
                                            op=Alu.mult)
                    nc.scalar.activation(mk[:], mk[:], Act.Copy,
                                         accum_out=sums[:, i:i + 1])
                # k=40 masked sum on DVE (junk out reuses marg, dead after
                # the Arctan read, which S40 transitively waits on via g)
                nc.vector.scalar_tensor_tensor(
                    out=marg[:], in0=dsb[:], scalar=maxv[:, 39:40],
                    in1=g[:], op0=Alu.is_ge, op1=Alu.mult,
                    accum_out=sums[:, 2:3])
                # margin count at rank 40: candidate-miss + (near-)tie flag
                thr40 = small.tile([128, 1], f32, tag="thr40")
                nc.vector.tensor_scalar(
                    out=thr40[:], in0=maxv[:, 39:40], scalar1=1.0 + CNT_REL,
                    scalar2=CNT_ABS, op0=Alu.mult, op1=Alu.subtract)
                nc.vector.tensor_scalar(
                    out=t[:], in0=dsb[:], scalar1=thr40[:, 0:1],
                    scalar2=None, op0=Alu.is_ge, op1=Alu.add,
                    accum_out=sums[:, 3:4])

                nc.sync.dma_start(out_o[bt, ro:ro + 128, :], outt[:])

    nc.compile()
    return nc


def _hamilton(a, b):
    aw, ax, ay, az = a[..., 0], a[..., 1], a[..., 2], a[..., 3]
    bw, bx, by, bz = b[..., 0], b[..., 1], b[..., 2], b[..., 3]
    return np.stack([
        aw * bw - ax * bx - ay * by - az * bz,
        aw * bx + ax * bw + ay * bz - az * by,
        aw * by - ax * bz + ay * bw + az * bx,
        aw * bz + ax * by - ay * bx + az * bw,
    ], axis=-1).astype(np.float32)


def _host_prep(points):
    """numpy fp32 mirror of the reference preprocessing."""
    xyz = points[..., :3]
    cent = ((xyz.min(axis=2) + xyz.max(axis=2)) * np.float32(0.5))
    d = (xyz - cent[:, :, None, :]).astype(np.float32)
    n = np.sqrt((d * d).sum(-1, keepdims=True)).astype(np.float32)
    d = (d / np.maximum(n, np.float32(1e-12))).astype(np.float32)
    dot = np.clip(d[..., 1], np.float32(-1.0 + 1e-7), np.float32(1.0 - 1e-7))
    half = (np.arccos(dot) * np.float32(0.5)).astype(np.float32)
    axis = np.stack([d[..., 2], np.zeros_like(dot), -d[..., 0]], -1)
    an = np.sqrt((axis * axis).sum(-1, keepdims=True)).astype(np.float32)
    axis = (axis / np.maximum(an, np.float32(1e-12))).astype(np.float32)
    s = np.sin(half).astype(np.float32)
    bq = np.stack([np.cos(half).astype(np.float32), axis[..., 0] * s,
                   axis[..., 1] * s, axis[..., 2] * s], -1).astype(np.float32)
    conj = np.array([1, -1, -1, -1], np.float32)
    qf = _hamilton(bq[:, 1:], bq[:, :-1] * conj)
    qn = np.sqrt((qf * qf).sum(-1, keepdims=True)).astype(np.float32)
    qf = (qf / np.maximum(qn, np.float32(1e-12))).astype(np.float32)
    src = np.ascontiguousarray(xyz[:, :-1])          # (B,T,P,3)
    sq = (src * src).sum(-1).astype(np.float32)      # (B,T,P)
    return src, sq, qf


def _device_inputs(src, sq, qf):
    srcf = src.reshape(NBT, P, 3)
    sqf = sq.reshape(NBT, P)
    qff = qf.reshape(NBT, P, 4)

    aug_rhs = np.empty((NBT, AUGK, P), np.float32)
    aug_rhs[:, 0:3] = srcf.transpose(0, 2, 1)
    aug_rhs[:, 3] = -1.0
    aug_rhs[:, 4] = -sqf
    qf_rhs = np.ascontiguousarray(qff.transpose(0, 2, 1))

    lhs_full = np.empty((NBT, AUGK, P), np.float32)
    lhs_full[:, 0:3] = 2.0 * srcf.transpose(0, 2, 1)
    lhs_full[:, 3] = sqf
    lhs_full[:, 4] = 1.0

    in_maps = []
    for c in range(NCORES):
        sl = slice(c * CH, (c + 1) * CH)
        tab = np.zeros((NBT, AUGK, 2 * (P + CH)), np.float32)
        tab[:, :, 0:P] = aug_rhs
        tab[:, :, P:P + CH] = lhs_full[:, :, sl]
        tab[:, 0:QK, P + CH:P + CH + P] = qf_rhs
        tab[:, 0:QK, 2 * P + CH:2 * P + 2 * CH] = qf_rhs[:, :, sl]
        in_maps.append({"tables": np.ascontiguousarray(tab)})
    return in_maps


def _run_device(in_maps, trace=False, trace_kwargs=None):
    global _PROG
    from concourse.bass_utils import run_bass_kernel_spmd
    if _PROG is None:
        _PROG = _build_program()
    kw = dict(trace_kwargs or {})
    res = run_bass_kernel_spmd(_PROG, in_maps, core_ids=list(range(NCORES)),
                               trace=trace, **kw)
    return res


def _host_post(results, src, sq, qf):
    # reassemble per-core outputs -> (NBT, P, .)
    sums = np.empty((NBT, P, 4), np.float32)
    maxv = np.empty((NBT, P, KMAX), np.float32)
    for c, r in enumerate(results):
        sl = slice(c * CH, (c + 1) * CH)
        o = r["out"].reshape(NBT, CH, 4 + KMAX)
        sums[:, sl] = o[:, :, 0:4]
        maxv[:, sl] = o[:, :, 4:4 + KMAX]

    # per-(b,t,p) topk means of geo; device g = arccos/2 -> geo-sum = 4*g-sum
    mean_tk = np.empty((3, NBT, P), np.float32)
    for i, k in enumerate(SCALES):
        mean_tk[i] = sums[:, :, i] * np.float32(4.0 / k)
    cnt = sums[:, :, 3]

    # flag rows where a rank boundary is ambiguous (ties / near-ties /
    # possible candidate miss, via the device-side margin count)
    gap5 = maxv[:, :, 4] - maxv[:, :, 5]
    gap15 = maxv[:, :, 14] - maxv[:, :, 15]
    thr5 = (GAP_ABS + GAP_REL * np.abs(maxv[:, :, 4])).astype(np.float32)
    thr15 = (GAP_ABS + GAP_REL * np.abs(maxv[:, :, 14])).astype(np.float32)
    flags = (gap5 < thr5) | (gap15 < thr15) | (cnt > 40.5)
    fbt, fp_ = np.nonzero(flags)
    if len(fbt) > 0:
        srcf = src.reshape(NBT, P, 3)
        sqf = sq.reshape(NBT, P)
        qff = qf.reshape(NBT, P, 4)
        for bt in np.unique(fbt):
            pids = fp_[fbt == bt]
            Dr = (2.0 * (srcf[bt][pids] @ srcf[bt].T)
                  - sqf[bt][pids][:, None] - sqf[bt][None, :]).astype(np.float32)
            cidx = np.argpartition(-Dr, 59, axis=1)[:, :60]
            cidx = np.sort(cidx, axis=1)
            cvals = np.take_along_axis(Dr, cidx, axis=1)
            order = np.argsort(-cvals, axis=1, kind="stable")[:, :KMAX]
            top = np.take_along_axis(cidx, order, axis=1)
            nbr = qff[bt][top]                               # (n, 40, 4)
            dots = np.abs(np.einsum("nd,nkd->nk", qff[bt][pids],
                                    nbr)).astype(np.float32)
            dots = np.clip(dots, np.float32(0.0), np.float32(1.0 - 1e-7))
            geo = (2.0 * np.arccos(dots)).astype(np.float32)
            for i, k in enumerate(SCALES):
                mean_tk[i, bt, pids] = geo[:, :k].mean(axis=1,
                                                       dtype=np.float32)

    # mean over transitions -> (3, B, P)
    mean_inc = mean_tk.reshape(3, B, T, P).mean(axis=2, dtype=np.float32)

    out = np.empty((B, 3, F * P), np.float32)
    for i in range(3):
        mi = mean_inc[i]
        flat = np.sort(mi.reshape(-1), kind="stable")
        scale = np.float32(max(flat[(flat.size - 1) // 2], np.float32(1e-6)))
        if mi.max() > 0:
            rig = np.exp(-mi / scale).astype(np.float32)
        else:
            rig = np.ones_like(mi)
        out[:, i, :] = np.broadcast_to(rig[:, None, :], (B, F, P)).reshape(B, F * P)
    return out


def kernel(points_4d, num_frames=None, _trace=False, _trace_kwargs=None):
    points = np.asarray(points_4d, dtype=np.float32)
    assert points.shape == (B, F, P, 4)
    src, sq, qf = _host_prep(points)
    in_maps = _device_inputs(src, sq, qf)
    res = _run_device(in_maps, trace=_trace, trace_kwargs=_trace_kwargs)
    out = _host_post(res.results, src, sq, qf)
    kernel._last_result = res
    return out



# revision 3
# speedup vs baseline: 5.0715x; 5.0715x over previous
"""Trainium2 Bass kernel for nn_BearingQCCFeatureMotion (v6).

Pipeline (B=2, F=8, P=2048, SCALES=(5,15,40)):
  - host (numpy fp32, mirrors the reference formulas): bearing quaternions,
    per-transition relative quaternions q_fwd, squared norms; builds augmented
    matmul tables so the device computes both the pairwise neg-distance matrix
    D[p,q] = 2<x_p,x_q> - |x_p|^2 - |x_q|^2 and the quaternion-dot matrix
    Q[p,q] = <q_fwd[p], q_fwd[q]> as K=5 / K=4 matmuls.
  - device (8 cores, data-parallel over the P dimension; each core handles a
    256-row chunk of all 14 (b,t) transitions = 28 tiles of 128x2048):
      PE  : D and Q matmuls into PSUM
      ACT : PSUM->SBUF moves (copy / |Q|), the arccos factors
            s' = |c3|*sqrt(1-x) and w = (x+h)^2, and the rank-40 margin
            count as Sign(D - thr40'') with a free row-accumulator
      DVE : segment-candidate top-40 (16x max8 over 128-col segments ->
            128 candidates -> exact top-40 of candidates via 5x max8 +
            4x match_replace), the factored-cubic products
            p = (x-r)*s' and g = (w+d)*p = -arccos(|Q|)  [A&S 4.4.45,
            |err| <= 6.8e-5], and all three masked top-k sums as single
            is_ge*g passes with accum_out
      (GPSIMD is deliberately UNUSED: measured ~30us per [128,2048] pass on
       this stack, ~17x the cost-model figure -- it was v5's bottleneck.)
    Flag logic: near-ties at ranks 5/15 are gap-checked on the host from
    maxv; candidate misses and (near-)ties at rank 40 are caught by the
    sign-sum margin count. Flagged rows (~4%) are recomputed on the host.
  - host: means over transitions, flagged-row repair, lower-median scale,
    exp, broadcast to the (B, 3, F*P) output.
"""

import numpy as np

B, F, P = 2, 8, 2048
T = F - 1
NBT = B * T            # 14 independent (b, t) transitions
NCORES = 8
CH = P // NCORES       # 256 rows per core
SCALES = (5, 15, 40)
AUGK = 5               # [2x, 2y, 2z, sq, 1] . [x', y', z', -1, -sq']
QK = 4

SEG = 16               # stage1 segments per row
SEGW = P // SEG        # 128 columns per segment
KMAX = 40

# arccos(x) ~= |C3| * sqrt(1-x) * (R - x) * ((x + H)^2 + D0), 0<=x<=1
# (factored Abramowitz-Stegun 4.4.45 cubic; device computes the negative)
C3 = 0.018729300
R_ = 5.028992295
H_ = 0.532014154
D0 = 16.393222202
XSCALE = 1.0 - 1e-6    # |Q| scale guard so 1 - x stays >= 0 under fp error

# Margins: device D vs reference D divergence is a few fp32 ulps of the
# O(10) matmul terms (~3e-6 absolute); 1e-5 gives ~3x headroom.
CNT_REL = 1e-5
CNT_ABS = 1e-5
GAP_REL = 1e-5
GAP_ABS = 1e-5

_PROG = None


def _build_program(repeat=1, hw_loop_n=None):
    """Build + compile the per-core Bass/Tile program (same for all cores).

    hw_loop_n wraps the whole tile sequence in a device-side For_i loop
    (used for slope-based timing; outputs are just overwritten each pass).
    """
    from contextlib import ExitStack
    import concourse.tile as tile
    from concourse import bacc, mybir
    import concourse.bass as bass

    f32 = mybir.dt.float32
    f16 = mybir.dt.float16
    Alu = mybir.AluOpType
    Act = mybir.ActivationFunctionType

    nc = bacc.Bacc("TRN2", target_bir_lowering=False, debug=False)

    # register float activation-bias constants (one-time [128,1] memsets,
    # emitted into the preamble before the tile program / For_i loop)
    for cval in (C3 * C3, H_):
        cten = nc.alloc_sbuf_tensor(f"const-f32-{cval}", [128, 1], f32)
        nc.vector.memset(cten.ap(), cval)
        nc.const_aps.aps[(f32, cval)] = cten.ap()

    # packed per-bt tables: [0:P]=aug_rhs, [P:P+CH]=aug_lhsT,
    # [P+CH:2P+CH]=qf_rhs (rows 0-3), [2P+CH:2P+2CH]=qf_lhsT (rows 0-3)
    tables = nc.dram_tensor("tables", [NBT, AUGK, 2 * (P + CH)], f32,
                            kind="ExternalInput").ap()
    # packed output: cols 0:4 = [S5, S15, S40, signsum40], 4:44 = maxv
    out_o = nc.dram_tensor("out", [NBT, CH, 4 + KMAX], f32,
                           kind="ExternalOutput").ap()

    with tile.TileContext(nc) as tc, ExitStack() as ctx:
        tabs = ctx.enter_context(tc.tile_pool(name="tabs", bufs=2))
        psum_d = ctx.enter_context(tc.tile_pool(name="psd", bufs=1, space="PSUM"))
        psum_q = ctx.enter_context(tc.tile_pool(name="psq", bufs=1, space="PSUM"))
        work2 = ctx.enter_context(tc.tile_pool(name="work", bufs=2))
        small = ctx.enter_context(tc.tile_pool(name="small", bufs=3))

        def work(shape, dtype, tag):
            return work2.tile(shape, dtype, tag=tag, name=tag)

        if hw_loop_n is not None:
            ctx.enter_context(tc.For_i(0, hw_loop_n, 1))

        for bt in [i % NBT for i in range(NBT * repeat)]:
            # per-(b,t) tables, one packed DMA; all matmul operand slices
            # start at partition 0
            tab = tabs.tile([AUGK, 2 * (P + CH)], f32, tag="tab")
            nc.sync.dma_start(tab[:], tables[bt])
            a_rhs = tab[:, 0:P]
            a_lhs = tab[:, P:P + CH]
            q_rhs = tab[0:QK, P + CH:P + CH + P]
            q_lhs = tab[0:QK, 2 * P + CH:2 * P + 2 * CH]

            for ch in range(CH // 128):
                ro = ch * 128

                psd = psum_d.tile([128, P], f32)
                for j in range(4):
                    nc.tensor.matmul(
                        psd[:, bass.ts(j, 512)],
                        lhsT=a_lhs[:, ro:ro + 128],
                        rhs=a_rhs[:, bass.ts(j, 512)],
                        start=True, stop=True)
                psq = psum_q.tile([128, P], f32)
                for j in range(4):
                    nc.tensor.matmul(
                        psq[:, bass.ts(j, 512)],
                        lhsT=q_lhs[:, ro:ro + 128],
                        rhs=q_rhs[:, bass.ts(j, 512)],
                        start=True, stop=True)

                # PSUM evictions (ACT). psd/psq each have exactly one
                # reader, so the PE can roll into the next tile's matmuls
                # as soon as these two passes finish.
                dsb = work([128, P], f32, "dsb")
                nc.scalar.copy(dsb[:], psd[:])
                x = work([128, P], f32, "x")
                nc.scalar.activation(x[:], psq[:], Act.Abs, scale=XSCALE)

                # top-40 of D per row (DVE), interleaved with the ACT
                # arccos-factor passes below by the schedulers.
                outt = small.tile([128, 4 + KMAX], f32, tag="outt")
                maxv = outt[:, 4:4 + KMAX]
                sums = outt[:, 0:4]
                # stage1: top-8 of each 128-col segment -> 128 candidates
                cand = small.tile([128, SEG * 8], f32, tag="cand")
                for s in range(SEG):
                    nc.vector.max(cand[:, 8 * s:8 * s + 8],
                                  dsb[:, SEGW * s:SEGW * (s + 1)])
                # stage2: exact top-40 of the candidates
                candw = small.tile([128, SEG * 8], f32, tag="candw")
                nc.vector.max(maxv[:, 0:8], cand[:])
                nc.vector.match_replace(candw[:], maxv[:, 0:8], cand[:], -1e30)
                for r in range(1, 5):
                    nc.vector.max(maxv[:, 8 * r:8 * r + 8], candw[:])
                    if r < 4:
                        nc.vector.match_replace(candw[:], maxv[:, 8 * r:8 * r + 8],
                                                candw[:], -1e30)

                # arccos factors: s' = |C3|*sqrt(1-x) (ACT), w = (x+H)^2
                # (ACT), p = (x-R)*s' (DVE), g = (w+D0)*p = -arccos(x) (DVE)
                sp = work([128, P], f16, "sp")
                nc.scalar.activation(sp[:], x[:], Act.Sqrt,
                                     scale=-C3 * C3, bias=C3 * C3)
                w = work([128, P], f16, "w")
                nc.scalar.activation(w[:], x[:], Act.Square, bias=H_)
                p = work([128, P], f16, "p")
                nc.vector.scalar_tensor_tensor(
                    out=p[:], in0=x[:], scalar=R_, in1=sp[:],
                    op0=Alu.subtract, op1=Alu.mult)
                g = work([128, P], f16, "g")
                nc.vector.scalar_tensor_tensor(
                    out=g[:], in0=w[:], scalar=D0, in1=p[:],
                    op0=Alu.add, op1=Alu.mult)

                # masked top-k sums: one DVE pass each, junk full-width out
                junk = work([128, P], f16, "junk")
                for i, col in ((0, 4), (1, 14), (2, 39)):
                    nc.vector.scalar_tensor_tensor(
                        out=junk[:], in0=dsb[:], scalar=maxv[:, col:col + 1],
                        in1=g[:], op0=Alu.is_ge, op1=Alu.mult,
                        accum_out=sums[:, i:i + 1])

                # margin count at rank 40 as a sign-sum (free ACT accum):
                # sum(sign(D - thr40'')) with thr40'' = thr40*(1+eps)-eps;
                # host reconstructs count = (P + signsum)/2.
                thr40n = small.tile([128, 1], f32, tag="thr40n")
                nc.vector.tensor_scalar(
                    out=thr40n[:], in0=maxv[:, 39:40],
                    scalar1=-(1.0 + CNT_REL), scalar2=CNT_ABS,
                    op0=Alu.mult, op1=Alu.add)
                sjunk = work([128, P], f16, "sjunk")
                nc.scalar.activation(sjunk[:], dsb[:], Act.Sign,
                                     bias=thr40n[:, 0:1],
                                     accum_out=sums[:, 3:4])

                nc.sync.dma_start(out_o[bt, ro:ro + 128, :], outt[:])

    nc.compile()
    return nc


def _hamilton(a, b):
    aw, ax, ay, az = a[..., 0], a[..., 1], a[..., 2], a[..., 3]
    bw, bx, by, bz = b[..., 0], b[..., 1], b[..., 2], b[..., 3]
    return np.stack([
        aw * bw - ax * bx - ay * by - az * bz,
        aw * bx + ax * bw + ay * bz - az * by,
        aw * by - ax * bz + ay * bw + az * bx,
        aw * bz + ax * by - ay * bx + az * bw,
    ], axis=-1).astype(np.float32)


def _host_prep(points):
    """numpy fp32 mirror of the reference preprocessing."""
    xyz = points[..., :3]
    cent = ((xyz.min(axis=2) + xyz.max(axis=2)) * np.float32(0.5))
    d = (xyz - cent[:, :, None, :]).astype(np.float32)
    n = np.sqrt((d * d).sum(-1, keepdims=True)).astype(np.float32)
    d = (d / np.maximum(n, np.float32(1e-12))).astype(np.float32)
    dot = np.clip(d[..., 1], np.float32(-1.0 + 1e-7), np.float32(1.0 - 1e-7))
    half = (np.arccos(dot) * np.float32(0.5)).astype(np.float32)
    axis = np.stack([d[..., 2], np.zeros_like(dot), -d[..., 0]], -1)
    an = np.sqrt((axis * axis).sum(-1, keepdims=True)).astype(np.float32)
    axis = (axis / np.maximum(an, np.float32(1e-12))).astype(np.float32)
    s = np.sin(half).astype(np.float32)
    bq = np.stack([np.cos(half).astype(np.float32), axis[..., 0] * s,
                   axis[..., 1] * s, axis[..., 2] * s], -1).astype(np.float32)
    conj = np.array([1, -1, -1, -1], np.float32)
    qf = _hamilton(bq[:, 1:], bq[:, :-1] * conj)
    qn = np.sqrt((qf * qf).sum(-1, keepdims=True)).astype(np.float32)
    qf = (qf / np.maximum(qn, np.float32(1e-12))).astype(np.float32)
    src = np.ascontiguousarray(xyz[:, :-1])          # (B,T,P,3)
    sq = (src * src).sum(-1).astype(np.float32)      # (B,T,P)
    return src, sq, qf


def _device_inputs(src, sq, qf):
    srcf = src.reshape(NBT, P, 3)
    sqf = sq.reshape(NBT, P)
    qff = qf.reshape(NBT, P, 4)

    aug_rhs = np.empty((NBT, AUGK, P), np.float32)
    aug_rhs[:, 0:3] = srcf.transpose(0, 2, 1)
    aug_rhs[:, 3] = -1.0
    aug_rhs[:, 4] = -sqf
    qf_rhs = np.ascontiguousarray(qff.transpose(0, 2, 1))

    lhs_full = np.empty((NBT, AUGK, P), np.float32)
    lhs_full[:, 0:3] = 2.0 * srcf.transpose(0, 2, 1)
    lhs_full[:, 3] = sqf
    lhs_full[:, 4] = 1.0

    in_maps = []
    for c in range(NCORES):
        sl = slice(c * CH, (c + 1) * CH)
        tab = np.zeros((NBT, AUGK, 2 * (P + CH)), np.float32)
        tab[:, :, 0:P] = aug_rhs
        tab[:, :, P:P + CH] = lhs_full[:, :, sl]
        tab[:, 0:QK, P + CH:P + CH + P] = qf_rhs
        tab[:, 0:QK, 2 * P + CH:2 * P + 2 * CH] = qf_rhs[:, :, sl]
        in_maps.append({"tables": np.ascontiguousarray(tab)})
    return in_maps


def _run_device(in_maps, trace=False, trace_kwargs=None):
    global _PROG
    from concourse.bass_utils import run_bass_kernel_spmd
    if _PROG is None:
        _PROG = _build_program()
    kw = dict(trace_kwargs or {})
    res = run_bass_kernel_spmd(_PROG, in_maps, core_ids=list(range(NCORES)),
                               trace=trace, **kw)
    return res


def _host_post(results, src, sq, qf):
    # reassemble per-core outputs -> (NBT, P, .)
    sums = np.empty((NBT, P, 4), np.float32)
    maxv = np.empty((NBT, P, KMAX), np.float32)
    for c, r in enumerate(results):
        sl = slice(c * CH, (c + 1) * CH)
        o = r["out"].reshape(NBT, CH, 4 + KMAX)
        sums[:, sl] = o[:, :, 0:4]
        maxv[:, sl] = o[:, :, 4:4 + KMAX]

    # per-(b,t,p) topk means of geo; device g = -arccos -> geo-sum = -2*g-sum
    mean_tk = np.empty((3, NBT, P), np.float32)
    for i, k in enumerate(SCALES):
        mean_tk[i] = sums[:, :, i] * np.float32(-2.0 / k)
    # sign-sum -> count of D >= thr40'' (exact-equal contributes 1/2)
    cnt = (np.float32(P) + sums[:, :, 3]) * np.float32(0.5)

    # flag rows where a rank boundary is ambiguous (ties / near-ties /
    # possible candidate miss, via the device-side margin count)
    gap5 = maxv[:, :, 4] - maxv[:, :, 5]
    gap15 = maxv[:, :, 14] - maxv[:, :, 15]
    thr5 = (GAP_ABS + GAP_REL * np.abs(maxv[:, :, 4])).astype(np.float32)
    thr15 = (GAP_ABS + GAP_REL * np.abs(maxv[:, :, 14])).astype(np.float32)
    flags = (gap5 < thr5) | (gap15 < thr15) | (cnt > 40.4)
    fbt, fp_ = np.nonzero(flags)
    if len(fbt) > 0:
        srcf = src.reshape(NBT, P, 3)
        sqf = sq.reshape(NBT, P)
        qff = qf.reshape(NBT, P, 4)
        for bt in np.unique(fbt):
            pids = fp_[fbt == bt]
            Dr = (2.0 * (srcf[bt][pids] @ srcf[bt].T)
                  - sqf[bt][pids][:, None] - sqf[bt][None, :]).astype(np.float32)
            cidx = np.argpartition(-Dr, 59, axis=1)[:, :60]
            cidx = np.sort(cidx, axis=1)
            cvals = np.take_along_axis(Dr, cidx, axis=1)
            order = np.argsort(-cvals, axis=1, kind="stable")[:, :KMAX]
            top = np.take_along_axis(cidx, order, axis=1)
            nbr = qff[bt][top]                               # (n, 40, 4)
            dots = np.abs(np.einsum("nd,nkd->nk", qff[bt][pids],
                                    nbr)).astype(np.float32)
            dots = np.clip(dots, np.float32(0.0), np.float32(1.0 - 1e-7))
            geo = (2.0 * np.arccos(dots)).astype(np.float32)
            for i, k in enumerate(SCALES):
                mean_tk[i, bt, pids] = geo[:, :k].mean(axis=1,
                                                       dtype=np.float32)

    # mean over transitions -> (3, B, P)
    mean_inc = mean_tk.reshape(3, B, T, P).mean(axis=2, dtype=np.float32)

    out = np.empty((B, 3, F * P), np.float32)
    for i in range(3):
        mi = mean_inc[i]
        flat = np.sort(mi.reshape(-1), kind="stable")
        scale = np.float32(max(flat[(flat.size - 1) // 2], np.float32(1e-6)))
        if mi.max() > 0:
            rig = np.exp(-mi / scale).astype(np.float32)
        else:
            rig = np.ones_like(mi)
        out[:, i, :] = np.broadcast_to(rig[:, None, :], (B, F, P)).reshape(B, F * P)
    return out


def kernel(points_4d, num_frames=None, _trace=False, _trace_kwargs=None):
    points = np.asarray(points_4d, dtype=np.float32)
    assert points.shape == (B, F, P, 4)
    src, sq, qf = _host_prep(points)
    in_maps = _device_inputs(src, sq, qf)
    res = _run_device(in_maps, trace=_trace, trace_kwargs=_trace_kwargs)
    out = _host_post(res.results, src, sq, qf)
    kernel._last_result = res
    return out
